# revision 1
# baseline (speedup 1.0000x reference)
"""Trainium2 Bass kernel for NeighborhoodNormalization.

Math: the reference builds a per-point homogeneous transform
T = [[ux,-uy,0,px],[uy,ux,0,py],[0,0,1,pz],[0,0,0,1]] (u = p/||p||),
inverts it, and applies it to 64 neighbors per point.  The inverse has a
closed form: with r2 = px^2+py^2, n = ||p||, a = n/r2, cx = px*a, cy = py*a:

    out.x =  cx*qx + cy*qy + tx      tx = -(cx*px + cy*py)
    out.y = -cy*qx + cx*qy + ty      ty =  (cy*px - cx*py)
    out.z =  qz - pz

So the kernel is pure elementwise math (memory-bound).  Sharding: pure data
parallel over the N=8192 point axis across 8 cores (1024 points/core).

Per-core layout: 16384 points = 128 partitions x 128 columns, where
partition p = b*8 + s holds points with local n = s*128 + t (t = column).
Neighbor rows (64*3 floats) stay contiguous in HBM per point, so DMAs are
[128 partitions x G*768B contiguous] blocks.  Per-point coefficients live as
[128,128] SBUF tiles; column t supplies the per-partition scalars for the
fused tensor_scalar / scalar_tensor_tensor / activation ops of column t.
"""

import sys

if "/opt/trn_rl_repo" not in sys.path:
    sys.path.insert(0, "/opt/trn_rl_repo")

import numpy as np

import concourse.bass as bass
import concourse.bacc as bacc
import concourse.mybir as mybir
from concourse.tile import TileContext
from concourse.bass_utils import run_bass_kernel_spmd

B = 16
N = 8192
K = 64
NCORES = 8
NLOC = N // NCORES  # 1024 points per core
P = 128             # SBUF partitions
S = NLOC // P       # 8 partition sub-blocks per batch entry
T = (B * NLOC) // P  # 128 point-columns per partition
G = 16              # columns per DMA group
NG = T // G

F32 = mybir.dt.float32
OP = mybir.AluOpType
AF = mybir.ActivationFunctionType

_CACHE = {}


def _build_nc():
    nc = bacc.Bacc(None, target_bir_lowering=False)

    pts = nc.declare_dram_parameter("points", [B, NLOC, 3], F32, isOutput=False)
    nb = nc.declare_dram_parameter("neighborhoods", [B, NLOC, K, 3], F32, isOutput=False)
    out = nc.declare_dram_parameter("out", [B, NLOC, K, 3], F32, isOutput=True)

    # partition = (b s), columns = t, free = 192 floats per point
    nbr = nb[:].rearrange("b (s t) k c -> (b s) t (k c)", s=S)
    outr = out[:].rearrange("b (s t) k c -> (b s) t (k c)", s=S)
    ptsr = pts[:].rearrange("b (s t) c -> (b s) (t c)", s=S)

    with TileContext(nc) as tc:
        with tc.tile_pool(name="const", bufs=1) as cpool, \
             tc.tile_pool(name="io_in", bufs=6) as inpool, \
             tc.tile_pool(name="io_out", bufs=6) as outpool, \
             tc.tile_pool(name="tmp", bufs=16) as tmppool:

            pts_sb = cpool.tile([P, T * 3], F32, tag="pts")
            nc.sync.dma_start(out=pts_sb[:], in_=ptsr)
            pv = pts_sb[:].rearrange("p (t c) -> p t c", c=3)
            px = pv[:, :, 0]
            py = pv[:, :, 1]
            pz = pv[:, :, 2]

            def ctile(tag):
                return cpool.tile([P, T], F32, tag=tag, name=tag)

            t1 = ctile("t1")
            t2 = ctile("t2")
            r2 = ctile("r2")
            n2 = ctile("n2")
            nn = ctile("nn")
            ir2 = ctile("ir2")
            aa = ctile("aa")
            cx = ctile("cx")
            cy = ctile("cy")
            ncy = ctile("ncy")
            tx = ctile("tx")
            ty = ctile("ty")
            npz = ctile("npz")

            nc.vector.tensor_mul(out=t1[:], in0=px, in1=px)
            nc.vector.tensor_mul(out=t2[:], in0=py, in1=py)
            nc.vector.tensor_add(out=r2[:], in0=t1[:], in1=t2[:])
            nc.vector.tensor_mul(out=t1[:], in0=pz, in1=pz)
            nc.vector.tensor_add(out=n2[:], in0=r2[:], in1=t1[:])
            nc.scalar.sqrt(out=nn[:], in_=n2[:])
            nc.vector.reciprocal(out=ir2[:], in_=r2[:])
            nc.vector.tensor_mul(out=aa[:], in0=nn[:], in1=ir2[:])
            nc.vector.tensor_mul(out=cx[:], in0=px, in1=aa[:])
            nc.vector.tensor_mul(out=cy[:], in0=py, in1=aa[:])
            nc.vector.tensor_scalar_mul(out=ncy[:], in0=cy[:], scalar1=-1.0)
            # tx = -(cx*px + cy*py)
            nc.vector.tensor_mul(out=t1[:], in0=cx[:], in1=px)
            nc.vector.tensor_mul(out=t2[:], in0=cy[:], in1=py)
            nc.vector.tensor_add(out=t1[:], in0=t1[:], in1=t2[:])
            nc.vector.tensor_scalar_mul(out=tx[:], in0=t1[:], scalar1=-1.0)
            # ty = cy*px - cx*py
            nc.vector.tensor_mul(out=t1[:], in0=cy[:], in1=px)
            nc.vector.tensor_mul(out=t2[:], in0=cx[:], in1=py)
            nc.vector.tensor_sub(out=ty[:], in0=t1[:], in1=t2[:])
            nc.vector.tensor_scalar_mul(out=npz[:], in0=pz, scalar1=-1.0)

            for g in range(NG):
                nb_t = inpool.tile([P, G, K, 3], F32, tag="nb", name=f"nb{g}")
                nc.sync.dma_start(
                    out=nb_t[:].rearrange("p g k c -> p g (k c)"),
                    in_=nbr[:, g * G:(g + 1) * G, :],
                )
                ot = outpool.tile([P, G, K, 3], F32, tag="ot", name=f"ot{g}")
                # out.z = qz - pz for the whole group in one wide op
                # (npz broadcast along the K axis via 0-stride AP)
                npz_b = npz[:, g * G:(g + 1) * G, None].broadcast_to([P, G, K])
                nc.vector.tensor_add(
                    out=ot[:, :, :, 2], in0=nb_t[:, :, :, 2], in1=npz_b,
                )
                for i in range(G):
                    t = g * G + i
                    qx = nb_t[:, i, :, 0]
                    qy = nb_t[:, i, :, 1]
                    ox = ot[:, i, :, 0]
                    oy = ot[:, i, :, 1]
                    cx_t = cx[:, t:t + 1]
                    cy_t = cy[:, t:t + 1]
                    ncy_t = ncy[:, t:t + 1]
                    tx_t = tx[:, t:t + 1]
                    ty_t = ty[:, t:t + 1]

                    # i2 = cy*qy + tx   (ACT: Identity(in*scale + bias))
                    tmp1 = tmppool.tile([P, K], F32, tag="tmp1", name=f"tmp1_{t}")
                    nc.scalar.activation(
                        out=tmp1[:], in_=qy, func=AF.Identity,
                        bias=tx_t, scale=cy_t,
                    )
                    # out.x = cx*qx + i2
                    nc.vector.scalar_tensor_tensor(
                        out=ox, in0=qx, scalar=cx_t, in1=tmp1[:],
                        op0=OP.mult, op1=OP.add,
                    )
                    # j2 = -cy*qx + ty   (GpSimd: otherwise idle)
                    tmp2 = tmppool.tile([P, K], F32, tag="tmp2", name=f"tmp2_{t}")
                    nc.gpsimd.tensor_scalar(
                        out=tmp2[:], in0=qx, scalar1=ncy_t, scalar2=ty_t,
                        op0=OP.mult, op1=OP.add,
                    )
                    # out.y = cx*qy + j2
                    nc.vector.scalar_tensor_tensor(
                        out=oy, in0=qy, scalar=cx_t, in1=tmp2[:],
                        op0=OP.mult, op1=OP.add,
                    )
                # out-DMA on the ACT HWDGE ring so it overlaps the SP-ring
                # input stream (HWDGE is FIFO per issuing engine).
                nc.scalar.dma_start(
                    out=outr[:, g * G:(g + 1) * G, :],
                    in_=ot[:].rearrange("p g k c -> p g (k c)"),
                )

    nc.compile()
    return nc


def _get_nc():
    if "nc" not in _CACHE:
        _CACHE["nc"] = _build_nc()
    return _CACHE["nc"]


def kernel(points, neighborhoods):
    pts = np.ascontiguousarray(np.asarray(points, dtype=np.float32))
    nb = np.ascontiguousarray(np.asarray(neighborhoods, dtype=np.float32))
    assert pts.shape == (B, N, 3), pts.shape
    assert nb.shape == (B, N, K, 3), nb.shape

    in_maps = []
    for c in range(NCORES):
        sl = slice(c * NLOC, (c + 1) * NLOC)
        in_maps.append({
            "points": np.ascontiguousarray(pts[:, sl]),
            "neighborhoods": np.ascontiguousarray(nb[:, sl]),
        })

    res = run_bass_kernel_spmd(_get_nc(), in_maps, list(range(NCORES))).results
    out = np.concatenate([res[c]["out"] for c in range(NCORES)], axis=1)
    return out



# revision 3
# speedup vs baseline: 1.5796x; 1.5796x over previous
"""Trainium2 Bass kernel for NeighborhoodNormalization.

Math: the reference builds a per-point homogeneous transform
T = [[ux,-uy,0,px],[uy,ux,0,py],[0,0,1,pz],[0,0,0,1]] (u = p/||p||),
inverts it, and applies it to 64 neighbors per point.  Closed form with
r2 = px^2+py^2, n = ||p||, a = n/r2, cx = px*a, cy = py*a, d = q - p:

    out.x =  cx*dx + cy*dy
    out.y = -cy*dx + cx*dy
    out.z =  dz

Strategy (memory-bound, tolerance 2e-2 allows bf16):
  * Host converts neighborhoods to planar bf16 [3, B, Nloc, K] per core and
    upcasts/reinterleaves the bf16 output -> halves HBM traffic.
  * d = q - p is computed BY THE DMA: ACT prefills each SBUF group tile with
    -p (broadcast over K), then a SWDGE accum DMA (accum_op=add) streams the
    neighbor planes on top.  z needs no compute instruction at all.
  * The 4 products + 2 adds per group run as wide step-1 bf16 tensor_tensor
    ops on DVE; per-point coefficients enter as pair-duplicated [P,T,2] bf16
    tiles viewed with (G:2)(K/2:0)(2:1) broadcast APs so the 16-bit 2x perf
    mode still applies.
  * ox/oy overwrite dx/dy in the same tile (same-engine program order makes
    that safe), so one DMA per group moves all three output planes out.

Sharding: pure data parallel over N=8192 points across 8 cores.
Per-core layout: 16384 points = 128 partitions x 128 columns, partition
p = b*8 + s holds points with local n = s*128 + t.
"""

import sys

if "/opt/trn_rl_repo" not in sys.path:
    sys.path.insert(0, "/opt/trn_rl_repo")

import numpy as np
from ml_dtypes import bfloat16

import concourse.bass as bass
import concourse.bacc as bacc
import concourse.mybir as mybir
from concourse.tile import TileContext
from concourse.bass_utils import run_bass_kernel_spmd

B = 16
N = 8192
K = 64
NCORES = 8
NLOC = N // NCORES  # 1024 points per core
P = 128             # SBUF partitions
S = NLOC // P       # 8 partition sub-blocks per batch entry
T = (B * NLOC) // P  # 128 point-columns per partition
G = 16              # columns per group
NG = T // G

F32 = mybir.dt.float32
BF16 = mybir.dt.bfloat16
OP = mybir.AluOpType
AF = mybir.ActivationFunctionType

_CACHE = {}


def _build_nc():
    nc = bacc.Bacc(None, target_bir_lowering=False)

    pts = nc.declare_dram_parameter("points", [B, NLOC, 3], F32, isOutput=False)
    nb = nc.declare_dram_parameter("nbp", [3, B, NLOC, K], BF16, isOutput=False)
    out = nc.declare_dram_parameter("outp", [3, B, NLOC, K], BF16, isOutput=True)

    # partition = (b s), then [plane c, column t, neighbor k]
    nbr = nb[:].rearrange("c b (s t) k -> (b s) c t k", s=S)
    outr = out[:].rearrange("c b (s t) k -> (b s) c t k", s=S)
    ptsr = pts[:].rearrange("b (s t) c -> (b s) (t c)", s=S)

    with TileContext(nc) as tc:
        with tc.tile_pool(name="const", bufs=1) as cpool, \
             tc.tile_pool(name="io", bufs=1) as iopool, \
             tc.tile_pool(name="tmp", bufs=3) as tmppool:

            pts_sb = cpool.tile([P, T * 3], F32, tag="pts")
            nc.sync.dma_start(out=pts_sb[:], in_=ptsr)
            pv = pts_sb[:].rearrange("p (t c) -> p t c", c=3)
            px = pv[:, :, 0]
            py = pv[:, :, 1]
            pz = pv[:, :, 2]

            def ctile(tag, dtype=F32, shape=None):
                return cpool.tile(shape or [P, T], dtype, tag=tag, name=tag)

            t1 = ctile("t1")
            t2 = ctile("t2")
            r2 = ctile("r2")
            n2 = ctile("n2")
            nn = ctile("nn")
            ir2 = ctile("ir2")
            aa = ctile("aa")
            cx = ctile("cx")
            cy = ctile("cy")

            nc.vector.tensor_mul(out=t1[:], in0=px, in1=px)
            nc.vector.tensor_mul(out=t2[:], in0=py, in1=py)
            nc.vector.tensor_add(out=r2[:], in0=t1[:], in1=t2[:])
            nc.vector.tensor_mul(out=t1[:], in0=pz, in1=pz)
            nc.vector.tensor_add(out=n2[:], in0=r2[:], in1=t1[:])
            nc.scalar.sqrt(out=nn[:], in_=n2[:])
            nc.vector.reciprocal(out=ir2[:], in_=r2[:])
            nc.vector.tensor_mul(out=aa[:], in0=nn[:], in1=ir2[:])
            nc.vector.tensor_mul(out=cx[:], in0=px, in1=aa[:])
            nc.vector.tensor_mul(out=cy[:], in0=py, in1=aa[:])

            # pair-duplicated bf16 coefficient tiles: cxd[p,t,:] = (cx, cx)
            cxd = ctile("cxd", BF16, [P, T, 2])
            cyd = ctile("cyd", BF16, [P, T, 2])
            nc.vector.tensor_copy(cxd[:], cx[:, :, None].broadcast_to([P, T, 2]))
            nc.vector.tensor_copy(cyd[:], cy[:, :, None].broadcast_to([P, T, 2]))

            for g in range(NG):
                g0, g1 = g * G, (g + 1) * G
                # sections: 0 = dx->ox, 1 = dy->oy, 2 = oz
                sc = iopool.tile([P, 3, G, K], BF16, tag=f"sc{g}", name=f"sc{g}")

                # prefill sc[c, t, k] = -p_c[t]  (broadcast over k, cast bf16)
                pin = (
                    pv[:, g0:g1, :]
                    .rearrange("p t c -> p c t")[:, :, :, None]
                    .broadcast_to([P, 3, G, K])
                )
                nc.scalar.activation(out=sc[:], in_=pin, func=AF.Copy, scale=-1.0)

                # d = q - p via DMA accumulate (SWDGE CCE add)
                nc.gpsimd.dma_start(
                    out=sc[:], in_=nbr[:, :, g0:g1, :], accum_op=OP.add,
                )

                # [P, G, K/2, 2] views so the innermost step-1 bf16 pair keeps
                # the DVE in 2x mode even for the broadcast operand.
                dx4 = sc[:, 0].rearrange("p g (h two) -> p g h two", two=2)
                dy4 = sc[:, 1].rearrange("p g (h two) -> p g h two", two=2)
                bcx = cxd[:, g0:g1, None, :].broadcast_to([P, G, K // 2, 2])
                bcy = cyd[:, g0:g1, None, :].broadcast_to([P, G, K // 2, 2])

                def mtile(tag):
                    return tmppool.tile(
                        [P, G, K // 2, 2], BF16, tag=tag, name=f"{tag}_{g}"
                    )

                m1 = mtile("m1")
                m2 = mtile("m2")
                m3 = mtile("m3")
                m4 = mtile("m4")
                nc.vector.tensor_mul(out=m1[:], in0=dx4, in1=bcx)
                nc.vector.tensor_mul(out=m2[:], in0=dy4, in1=bcy)
                nc.vector.tensor_mul(out=m3[:], in0=dx4, in1=bcy)
                nc.vector.tensor_mul(out=m4[:], in0=dy4, in1=bcx)
                # ox/oy overwrite dx/dy (DVE executes in order; m3/m4 already
                # read them)
                nc.vector.tensor_add(out=dx4, in0=m1[:], in1=m2[:])
                nc.vector.tensor_sub(out=dy4, in0=m4[:], in1=m3[:])

                nc.sync.dma_start(out=outr[:, :, g0:g1, :], in_=sc[:])

    nc.compile()
    return nc


def _get_nc():
    if "nc" not in _CACHE:
        _CACHE["nc"] = _build_nc()
    return _CACHE["nc"]


def make_in_maps(points, neighborhoods):
    pts = np.ascontiguousarray(np.asarray(points, dtype=np.float32))
    nb = np.asarray(neighborhoods, dtype=np.float32)
    assert pts.shape == (B, N, 3), pts.shape
    assert nb.shape == (B, N, K, 3), nb.shape

    nb16 = nb.astype(bfloat16)  # [B, N, K, 3]

    in_maps = []
    for c in range(NCORES):
        sl = slice(c * NLOC, (c + 1) * NLOC)
        in_maps.append({
            "points": np.ascontiguousarray(pts[:, sl]),
            "nbp": np.ascontiguousarray(nb16[:, sl].transpose(3, 0, 1, 2)),
        })
    return in_maps


def kernel(points, neighborhoods):
    in_maps = make_in_maps(points, neighborhoods)
    res = run_bass_kernel_spmd(_get_nc(), in_maps, list(range(NCORES))).results
    outp = np.concatenate(
        [np.asarray(res[c]["outp"]) for c in range(NCORES)], axis=2
    )  # [3, B, N, K] bf16
    return np.ascontiguousarray(outp.transpose(1, 2, 3, 0)).astype(np.float32)
